# revision 1
# baseline (speedup 1.0000x reference)
"""Trainium2 Bass kernel for nn_EuESN_maml: assemble the 3N x 3N wave-equation
transition matrix A (N = 48*48) from c/dkx/dky fields.

A is all zeros except ~11 diagonals, so the kernel is a DMA memset of the
output plus strided diagonal scatter writes.

Sharding (SPMD, 8 cores): block-row index partitioned. Core k owns rows
[288k, 288k+288) of each of the three N-row block rows of A -> a [864, 6912]
shard per core. Each 288-row sub-band is written column-rotated by its first
global row index so diagonal positions are identical on every core (single
SPMD program); the host un-rotates with two slice copies while gathering.

Engine plan per core:
  vector: memset the zero tile (4 quarters), then the 11 diag value vectors
  sync  (HWDGE ring 0): 8 large contiguous ~3MB DMAs zero-fill the shard at
         ~355 GB/s (the ~358 GB/s per-core HBM cap); the first chunk goes in
         four quarter-DMAs chasing the quarter-memsets
  scalar(HWDGE ring 1): strided diagonal scatter DMAs, issued per sub-band as
         soon as the covering zero chunks have landed (overlaps the fill);
         sub-band 2 is split so only the last chunk's rows wait for fill end
"""

import math
import sys

import numpy as np

sys.path.insert(0, "/opt/trn_rl_repo")

import concourse.bass as bass
import concourse.mybir as mybir
from concourse.bass_utils import run_bass_kernel_spmd

# ---- problem constants (hardcoded from the nn_EuESN_maml spec) ----
n = 48
N = n * n            # 2304
M3 = 3 * N           # 6912 (output is M3 x M3)
NCORES = 8
B = N // NCORES      # 288 rows per sub-band
ROWS = 3 * B         # 864 rows per core shard
DT, CN, KP = 1.0, 0.1, 1e-4
MI = 1.0 / (1.0 / DT - KP / 2.0)          # 1/diagM (diagM is constant)
K0 = (1.0 / DT + KP / 2.0) * MI           # A00 diagonal value (constant)
DXC = (DT / CN) * math.sqrt(2.0)          # dx = DXC * max(c)

# zero-fill: NCHUNK contiguous chunks of [128, ZW] elements each
ZW = 5832
NCHUNK = (ROWS * M3) // (128 * ZW)        # 8
CHUNK_ROWS = 128 * ZW // M3               # 108 shard rows per chunk

# packed per-core input: [c.flat (N)] + 8 vectors of length B
PK = N + 8 * B

# number of chained vector-engine ops (vchain semaphore target)
NVOPS = 28

NSLOTS = 11

# scatter DMAs: (sub_band, col0, kind, slot/base)
# kind "s": one diagonal from value slot; kind "p": two ADJACENT diagonals
# from an interleaved 2*B region (contiguous source, 8-byte descriptors)
# sub 0 (rows of A00|A01|A02), rotation 288k
# sub 1 (A10|A11|0), rotation 2304+288k
# sub 2 (A20|0|A22), rotation 4608+288k
SCATTER = [
    (0, [(0, "s", 0),             # A00 diag: K0
         (N - n, "s", 1),         # A01 k=-n: w*mge
         (N, "s", 2),             # A01 main: -w
         (2 * N - 1, "p", 3)]),   # A02 k=-1 (w*mmod) | A02 main (-w)
    (1, [(0, "s", 5),             # A11 diag
         (2 * N, "s", 6),         # A10 main: rx
         (2 * N + n, "s", 7)]),   # A10 k=+n: rx*mltn
    (2, [(0, "s", 8),             # A22 diag
         (N, "p", 9)]),           # A20 main (ry) | A20 k=+1 (ry*mmodn)
]
NSCATTER = 13

# zero chunks covering sub-band b: rows [288b, 288b+288)
def _cover_end(bnd):
    return -(-(288 * (bnd + 1)) // CHUNK_ROWS)  # ceil


def _build_program() -> bass.Bass:
    nc = bass.Bass()
    f32 = mybir.dt.float32

    pk = nc.declare_dram_parameter("pk", [1, PK], f32, isOutput=False)
    out = nc.declare_dram_parameter("out", [ROWS, M3], f32, isOutput=True)

    with (
        nc.sbuf_tensor([128, ZW], f32) as zt,          # zero tile
        nc.sbuf_tensor([1, PK], f32) as pkb,           # packed inputs
        nc.sbuf_tensor([1, NSLOTS * B], f32) as vv,    # diagonal value vectors
        nc.sbuf_tensor([1, 6 * B], f32) as tmp,        # scratch
        nc.sbuf_tensor([1, 8], f32) as scal,           # scalars
        nc.semaphore("in_sem") as in_sem,
        nc.semaphore("vchain") as vchain,
        nc.semaphore("zsem0") as zsem0,
        nc.semaphore("zsem1") as zsem1,
        nc.semaphore("zsem2") as zsem2,
        nc.semaphore("zsem2a") as zsem2a,
        nc.semaphore("zsem3") as zsem3,
        nc.semaphore("ddma_sem") as ddma_sem,
        nc.Block() as block,
    ):
        # packed-input slices (offsets in elements)
        def pslice(i0, ln):
            return pkb[0:1, i0 : i0 + ln]

        call = pslice(0, N)
        cv = pslice(N, B)
        dkxv = pslice(N + B, B)
        dkyv = pslice(N + 2 * B, B)
        iv = pslice(N + 3 * B, B)
        mge = pslice(N + 4 * B, B)
        mmod = pslice(N + 5 * B, B)
        mltn = pslice(N + 6 * B, B)
        mmodn = pslice(N + 7 * B, B)

        def vslot(s):
            return vv[0:1, s * B : (s + 1) * B]

        def vpair(s, off):
            # stride-2 view over the interleaved pair region at slot s
            return bass.AP(vv, s * B + off, [[NSLOTS * B, 1], [2, B]])

        def tslot(s):
            return tmp[0:1, s * B : (s + 1) * B]

        def sc(i):
            return scal[0:1, i : i + 1]

        mult = mybir.AluOpType.mult
        add = mybir.AluOpType.add

        # chunk -> zero-fill semaphore group: group b must cover all chunks
        # that touch sub-band b's rows and not yet belong to earlier groups
        ZSEMS = [zsem0, zsem1, zsem2, zsem3]
        ZGROUP = [0 if ci < _cover_end(0) else (1 if ci < _cover_end(1) else
                  (2 if ci < NCHUNK - 1 else 3)) for ci in range(NCHUNK)]
        ZGCOUNT = [ZGROUP.count(g) for g in range(4)]
        # the final chunk (108 rows) is split 81+27 so only the last 27
        # rows' scatter descriptors wait for the very end of the fill
        SLIV = ROWS - (NCHUNK - 1) * CHUNK_ROWS       # rows in last chunk
        MAIN2 = B - SLIV                              # sub2 rows before sliver
        W7A = 4374                                    # 81 rows
        S7A = 128 * W7A // M3
        S7B = SLIV - S7A                              # 27 rows

        # zsem increments per group: chunk 0 is issued as four quarter-DMAs
        # so its group gets 64 increments instead of 16
        ZINC = [64 * ZGROUP[:1].count(g) + 16 * ZGROUP[1:-1].count(g)
                for g in range(3)] + [16]

        @block.sync
        def _(sync):
            # zero-fill the whole shard from the (memset) zero tile; chunk 0
            # goes in four quarters chasing the DVE quarter-memsets, so fill
            # data starts ~1.3us after the vector engine boots
            Q = ZW // 4
            g0 = ZSEMS[ZGROUP[0]]
            for qi in range(4):
                sync.wait_ge(vchain, qi + 1)
                dst = bass.AP(out, qi * Q, [[ZW, 128], [1, Q]])
                sync.dma_start(dst, zt[:, qi * Q : (qi + 1) * Q]).then_inc(g0, 16)
            for ci in range(1, NCHUNK - 1):
                dst = bass.AP(out, ci * 128 * ZW, [[ZW, 128], [1, ZW]])
                g = ZGROUP[ci]
                sync.dma_start(dst, zt[:]).then_inc(ZSEMS[g], 16)
            off7 = (NCHUNK - 1) * 128 * ZW
            dst = bass.AP(out, off7, [[W7A, 128], [1, W7A]])
            sync.dma_start(dst, zt[:, :W7A]).then_inc(zsem2a, 16)
            dst = bass.AP(out, off7 + 128 * W7A, [[ZW - W7A, 128], [1, ZW - W7A]])
            sync.dma_start(dst, zt[:, W7A:]).then_inc(zsem3, 16)
            for g, zs in enumerate(ZSEMS):
                sync.wait_ge(zs, ZINC[g])
            sync.wait_ge(zsem2a, 16)

        @block.scalar
        def _(se):
            # input load + diagonal scatter, on the second HWDGE ring so the
            # scatter drains via the SDMA round-robin while the fill runs
            se.dma_start(pkb[:], pk[:]).then_inc(in_sem, 16)
            se.wait_ge(vchain, NVOPS)
            ndma = [0]

            def dodma(bnd, col0, kind, slot0, r0, cnt):
                off = bnd * B * M3 + col0 + r0 * (M3 + 1)
                if kind == "s":
                    dst = bass.AP(out, off, [[M3 + 1, cnt], [1, 1]])
                    src = vv[0:1, slot0 * B + r0 : slot0 * B + r0 + cnt]
                else:  # interleaved adjacent pair
                    dst = bass.AP(out, off, [[M3 + 1, cnt], [1, 2]])
                    src = vv[0:1,
                             slot0 * B + 2 * r0 : slot0 * B + 2 * (r0 + cnt)]
                se.dma_start(dst, src, single_packet=True).then_inc(
                    ddma_sem, 16)
                ndma[0] += 1

            with nc.allow_non_contiguous_dma(reason="diagonal scatter"):
                for bnd, segs in SCATTER:
                    for g in range(bnd + 1):
                        se.wait_ge(ZSEMS[g], ZINC[g])
                    for col0, kind, slot0 in segs:
                        dodma(bnd, col0, kind, slot0, 0,
                              MAIN2 if bnd == 2 else B)
                # sliver: sub-band 2 rows in the split final zero chunk
                se.wait_ge(zsem2a, 16)
                for col0, kind, slot0 in SCATTER[2][1]:
                    dodma(2, col0, kind, slot0, MAIN2, S7A)
                se.wait_ge(ZSEMS[3], ZINC[3])
                for col0, kind, slot0 in SCATTER[2][1]:
                    dodma(2, col0, kind, slot0, MAIN2 + S7A, S7B)
            assert ndma[0] == NSCATTER
            se.wait_ge(ddma_sem, 16 * ndma[0])

        @block.vector
        def _(v):
            # engines have no scoreboarding: serialize the dependent DVE chain
            # through vchain so each op's writeback lands before the next read
            cnt = [0]

            def step(ins):
                cnt[0] += 1
                ins.then_inc(vchain, 1)
                v.wait_ge(vchain, cnt[0])

            Q = ZW // 4
            for qi in range(4):
                cnt[0] += 1
                v.memset(zt[:, qi * Q : (qi + 1) * Q], 0.0).then_inc(vchain, 1)
            v.wait_ge(vchain, 4)
            v.wait_ge(in_sem, 16)
            # dx = 10*sqrt(2)*max(c); invdx = 1/dx
            step(v.reduce_max(sc(0), call, axis=mybir.AxisListType.X))
            step(v.tensor_scalar_mul(sc(1), sc(0), float(DXC)))
            step(v.reciprocal(sc(2), sc(1)))
            # w = mi * rvec ; rv = rvec
            step(v.tensor_scalar(tslot(0), cv, sc(2), float(MI), mult, mult))
            step(v.tensor_scalar_mul(tslot(1), cv, sc(2)))
            step(v.memset(vslot(0), float(K0)))             # A00 diag
            step(v.tensor_mul(vslot(1), tslot(0), mge))     # A01 k=-n
            step(v.tensor_scalar_mul(vslot(2), tslot(0), -1.0))  # A01 main
            step(v.tensor_mul(vpair(3, 0), tslot(0), mmod))      # A02 k=-1
            step(v.tensor_scalar_mul(vpair(3, 1), tslot(0), -1.0))  # A02 main
            # x pass: gx = dkxv*iv; A11 = (1-gx)/(1+gx); A10 = rv/(1+gx)
            step(v.tensor_mul(tslot(2), dkxv, iv))
            step(v.tensor_scalar(tslot(3), tslot(2), 1.0, None, add))
            step(v.reciprocal(tslot(4), tslot(3)))
            step(v.tensor_scalar(tslot(5), tslot(2), -1.0, 1.0, mult, add))
            step(v.tensor_mul(vslot(5), tslot(5), tslot(4)))  # A11 diag
            step(v.tensor_mul(vslot(6), tslot(1), tslot(4)))  # A10 main
            step(v.tensor_mul(vslot(7), vslot(6), mltn))      # A10 k=+n
            # y pass
            step(v.tensor_mul(tslot(2), dkyv, iv))
            step(v.tensor_scalar(tslot(3), tslot(2), 1.0, None, add))
            step(v.reciprocal(tslot(4), tslot(3)))
            step(v.tensor_scalar(tslot(5), tslot(2), -1.0, 1.0, mult, add))
            step(v.tensor_mul(vslot(8), tslot(5), tslot(4)))  # A22 diag
            step(v.tensor_mul(vpair(9, 0), tslot(1), tslot(4)))  # A20 main: ry
            step(v.tensor_mul(vpair(9, 1), vpair(9, 0), mmodn))  # A20 k=+1
            assert cnt[0] == NVOPS, cnt[0]

    return nc


_nc_cache = None


def _get_nc() -> bass.Bass:
    global _nc_cache
    if _nc_cache is None:
        _nc_cache = _build_program()
    return _nc_cache


def _make_in_maps(c, dkx, dky):
    c = np.ascontiguousarray(c, dtype=np.float32)
    cT = np.ascontiguousarray(c.T).reshape(-1)
    dkxT = np.ascontiguousarray(np.asarray(dkx, np.float32).T).reshape(-1)
    dkyT = np.ascontiguousarray(np.asarray(dky, np.float32).T).reshape(-1)
    j = np.arange(N)
    iv = ((j // n) / 2.0).astype(np.float32)
    mge = (j >= n).astype(np.float32)
    mmod = (j % n != 0).astype(np.float32)
    mltn = np.where(j < N - n, -1.0, 0.0).astype(np.float32)
    mmodn = np.where((j + 1) % n != 0, -1.0, 0.0).astype(np.float32)

    in_maps = []
    for k in range(NCORES):
        sl = slice(k * B, (k + 1) * B)
        pk = np.concatenate(
            [c.reshape(-1), cT[sl], dkxT[sl], dkyT[sl], iv[sl],
             mge[sl], mmod[sl], mltn[sl], mmodn[sl]]
        ).astype(np.float32)[None, :]
        assert pk.shape == (1, PK)
        in_maps.append({"pk": pk})
    return in_maps


def _assemble(shards) -> np.ndarray:
    A = np.zeros((M3, M3), dtype=np.float32)
    for k in range(NCORES):
        shard = shards[k]
        for b in range(3):
            g0 = b * N + k * B
            band = shard[b * B : (b + 1) * B]
            if g0:
                A[g0 : g0 + B, g0:] = band[:, : M3 - g0]
                A[g0 : g0 + B, :g0] = band[:, M3 - g0 :]
            else:
                A[:B, :] = band
    return A


def kernel(c, dkx, dky, _trace=False):
    in_maps = _make_in_maps(c, dkx, dky)
    res = run_bass_kernel_spmd(
        _get_nc(), in_maps, core_ids=list(range(NCORES)), trace=_trace
    )
    A = _assemble([res.results[k]["out"] for k in range(NCORES)])
    if _trace:
        return A, res
    return A

